# revision 5
# baseline (speedup 1.0000x reference)
"""Low-rank (CP rank-20) LSTM, T=20 steps, distributed over 8 TRN2 NeuronCores.

Sharding: data-parallel over batch (B=4096 -> 512 per core); the rank-20
factor matrices and the output head are replicated.

Per-core layout (everything pre-transposed on host so the contraction dim
always lands on SBUF partitions; no on-device transposes needed):
  x   -> [T, D, B_l]         (a_x matmul rhs slices [128, b])
  vt  -> [64, 4H]            rows 0:20 = V_ih.T, rows 32:52 = V_hh.T, rest 0
  u_*  native [D|H, R], chunked to [128, 4, R] on chip
  wt  -> W_out.T [H, DO], b_out -> [DO, 1]
Per-core output: y.T [DO, B_l].

Two interleaved batch streams of 256. ACT (ScalarE) is the bottleneck:
per half-step it runs [tanh(g_s), tanh(c_sb), sigmoid(i_s,f_s), sigmoid(o_s)]
where (i,f) share one paired PSUM tile [128,2,4,256] so their sigmoid is a
single free-2048 instruction (per-instruction overhead ~380ns dominates the
savings). PSUM budget is exactly 16KB/partition: pg 4KB + pif 8KB + po 4KB,
so the a_x/a_h matmuls write into pg's first bank (rows 0:20 / 32:52) right
after sigma(g) frees it, and ONE [52,256] DVE copy assembles aT per step;
the j=0 gate matmul then overwrites that corner last.
"""

import os
import sys
import time

import numpy as np

_TRN_REPO = "/opt/trn_rl_repo"
if os.path.isdir(_TRN_REPO) and _TRN_REPO not in sys.path:
    sys.path.insert(0, _TRN_REPO)

import ml_dtypes  # noqa: E402
import concourse.bass as bass  # noqa: E402
import concourse.tile as tile  # noqa: E402
from concourse import bacc, mybir  # noqa: E402
from concourse.bass_utils import run_bass_kernel_spmd  # noqa: E402

B, T, D, H, R, DO = 4096, 20, 512, 512, 20, 64
N_CORES = 8
BL = B // N_CORES          # 512 batch per core
NS = 2                     # interleaved batch streams per core
BS = BL // NS              # 256 batch per stream
KC = 4                     # 128-row chunks in D and H

F32 = mybir.dt.float32
BF16 = mybir.dt.bfloat16
AF = mybir.ActivationFunctionType

_NC_CACHE = None


def build_nc(reps=1):
    nc = bacc.Bacc("TRN2", target_bir_lowering=False, debug=False,
                   num_devices=N_CORES)

    x_ext = nc.dram_tensor("x", [T, D, BL], BF16, kind="ExternalInput")
    uih_ext = nc.dram_tensor("u_ih", [D, R], BF16, kind="ExternalInput")
    uhh_ext = nc.dram_tensor("u_hh", [H, R], BF16, kind="ExternalInput")
    vt_ext = nc.dram_tensor("vt", [64, 4 * H], BF16, kind="ExternalInput")
    wt_ext = nc.dram_tensor("wt", [H, DO], BF16, kind="ExternalInput")
    b_ext = nc.dram_tensor("b_out", [DO, 1], F32, kind="ExternalInput")
    out_ext = nc.dram_tensor("out", [DO, BL], F32, kind="ExternalOutput")

    with tile.TileContext(nc) as tc:
        with (
            tc.tile_pool(name="const", bufs=1) as const,
            tc.tile_pool(name="xs", bufs=4) as xpool,
            tc.tile_pool(name="acts", bufs=1) as gpool,
            tc.tile_pool(name="state", bufs=1) as state,
            tc.tile_pool(name="ps", bufs=1, space="PSUM") as pspool,
        ):
            # ---- replicated weights ----
            u_ih = const.tile([128, KC, R], BF16, tag="u_ih")
            u_hh = const.tile([128, KC, R], BF16, tag="u_hh")
            vt = const.tile([64, 4 * H], BF16, tag="vt")
            wt = const.tile([128, KC, DO], BF16, tag="wt")
            bb = const.tile([DO, 1], F32, tag="bb")

            # ---- persistent state ----
            hT = [state.tile([128, KC, BS], BF16, tag=f"h{s}", name=f"hT{s}")
                  for s in range(NS)]
            cT = [state.tile([128, KC, BS], BF16, tag=f"c{s}", name=f"cT{s}")
                  for s in range(NS)]
            aT = [state.tile([64, BS], BF16, tag=f"a{s}", name=f"aT{s}")
                  for s in range(NS)]

            import contextlib
            loop_cm = tc.For_i(0, reps, 1) if reps > 1 else contextlib.nullcontext()
            with loop_cm:
                emit_recurrence(nc, tc, locals())

    nc.compile()
    return nc


def emit_recurrence(nc, tc, env):
    xpool = env["xpool"]; gpool = env["gpool"]; pspool = env["pspool"]
    u_ih = env["u_ih"]; u_hh = env["u_hh"]; vt = env["vt"]; wt = env["wt"]
    bb = env["bb"]; hT = env["hT"]; cT = env["cT"]; aT = env["aT"]
    x_ext = env["x_ext"]; out_ext = env["out_ext"]

    for s in range(NS):
        nc.vector.memset(aT[s][:], 0.0)
        nc.vector.memset(cT[s][:], 0.0)

    xt = {}

    def load_x(t):
        xt[t] = xpool.tile([128, KC, BL], BF16, tag="xt", name=f"xt{t}")
        src = x_ext.ap()[t].rearrange("(k p) b -> p k b", p=128)
        if t == 0:
            # stream-0's half (b 0:256) first: it gates the pipeline ramp
            for k, eng in enumerate([nc.sync, nc.gpsimd, nc.scalar, nc.sync]):
                eng.dma_start(xt[t][:, k, 0:BS], src[:, k, 0:BS])
            nc.sync.dma_start(xt[t][:, :, BS:], src[:, :, BS:])
        else:
            eng = nc.sync if t % 2 == 0 else nc.gpsimd
            eng.dma_start(xt[t][:], src)

    # column bases of the gates inside vt's 4H columns
    CI, CF, CG, CO = 0, H, 2 * H, 3 * H

    def prep_gates(t, s, first):
        """Emit a_x/a_h matmuls into pg's corner bank, the aT copy, and all
        gate matmuls for (t, s). Returns {'g': pg, 'if': pif, 'o': po}."""
        pg = pspool.tile([128, KC, BS], F32, tag="pg", name=f"pg{t}_{s}")
        pif = pspool.tile([128, 2, KC, BS], F32, tag="pif", name=f"pif{t}_{s}")
        po = pspool.tile([128, KC, BS], F32, tag="po", name=f"po{t}_{s}")
        # a_x -> pg[0:20, 0, :]
        for k in range(KC):
            nc.tensor.matmul(pg[0:20, 0, :], u_ih[:, k, :],
                             xt[t][:, k, s * BS:(s + 1) * BS],
                             start=(k == 0), stop=(k == KC - 1))
        if not first:
            # a_h -> pg[32:52, 0, :]
            for k in range(KC):
                nc.tensor.matmul(pg[32:52, 0, :], u_hh[:, k, :], hT[s][:, k, :],
                                 start=(k == 0), stop=(k == KC - 1))
            nc.vector.tensor_copy(aT[s][0:52, :], pg[0:52, 0, :])
        else:
            # aT rows 32:52 stay 0 (h0 = 0); don't copy junk over them
            nc.vector.tensor_copy(aT[s][0:20, :], pg[0:20, 0, :])
        # gate matmuls (K=64 single pass); pg chunk j=0 LAST (it overwrites
        # the a-corner, so it must wait for the copy above)
        for j in (1, 2, 3, 0):
            nc.tensor.matmul(pg[:, j, :], vt[:, CG + j * 128: CG + (j + 1) * 128],
                             aT[s][:, :], start=True, stop=True)
        for gi, c0 in ((0, CI), (1, CF)):
            for j in range(KC):
                nc.tensor.matmul(pif[:, gi, j, :],
                                 vt[:, c0 + j * 128: c0 + (j + 1) * 128],
                                 aT[s][:, :], start=True, stop=True)
        for j in range(KC):
            nc.tensor.matmul(po[:, j, :], vt[:, CO + j * 128: CO + (j + 1) * 128],
                             aT[s][:, :], start=True, stop=True)
        return {"g": pg, "if": pif, "o": po}

    # ---- prologue ----
    nc.sync.dma_start(u_ih[:], env["uih_ext"].ap().rearrange("(k p) r -> p k r", p=128))
    load_x(0)
    nc.scalar.dma_start(vt[:], env["vt_ext"].ap())
    warm = gpool.tile([1, 1], F32, tag="warm", name="warm")
    nc.vector.memset(warm[:], 0.0)
    nc.scalar.activation(warm[:], warm[:], AF.Sigmoid)
    nc.gpsimd.dma_start(u_hh[:], env["uhh_ext"].ap().rearrange("(k p) r -> p k r", p=128))
    nc.gpsimd.dma_start(wt[:], env["wt_ext"].ap().rearrange("(k p) o -> p k o", p=128))
    nc.gpsimd.dma_start(bb[:], env["b_ext"].ap())

    gate_ps = [None] * NS
    thc_pend = [None] * NS
    tg = [None] * NS
    sif = [None] * NS
    so = [None] * NS

    gate_ps[0] = prep_gates(0, 0, first=True)
    load_x(1)
    load_x(2)

    # ---- halfstep blocks ----
    for u in range(2 * T):
        t, s = u // 2, u % 2
        sb = 1 - s
        t_next = (u + 1) // 2   # step whose gates this block preps for sb

        # ACT 1: tanh(g_s)
        tg[s] = gpool.tile([128, KC, BS], BF16, tag=f"tg{s}", name=f"tg{s}")
        nc.scalar.activation(tg[s][:], gate_ps[s]["g"][:], AF.Tanh)

        # ACT 2: tanh(c_sb); DVE: h_sb = o_sb * tanh(c_sb)
        if thc_pend[sb] is not None:
            thc = gpool.tile([128, KC, BS], BF16, tag=f"th{sb}", name=f"thc{sb}")
            nc.scalar.activation(thc[:], cT[sb][:], AF.Tanh)
            nc.vector.tensor_mul(hT[sb][:], thc_pend[sb][:], thc[:])
            thc_pend[sb] = None

        # PE+DVE: prep sb's next gates (a_x, a_h, copy, gate MMs)
        if u + 1 < 2 * T:
            gate_ps[sb] = prep_gates(t_next, sb, first=(t_next == 0))

        # ACT 3: sigmoid(i_s, f_s) in one paired instruction
        sif[s] = gpool.tile([128, 2, KC, BS], BF16, tag=f"sif{s}", name=f"sif{s}")
        nc.scalar.activation(sif[s][:], gate_ps[s]["if"][:], AF.Sigmoid)
        # ACT 4: sigmoid(o_s)
        so[s] = gpool.tile([128, KC, BS], BF16, tag=f"so{s}", name=f"so{s}")
        nc.scalar.activation(so[s][:], gate_ps[s]["o"][:], AF.Sigmoid)

        # DVE cell: c_s = sf*c_s + si*tg
        nc.vector.tensor_mul(cT[s][:], sif[s][:, 1], cT[s][:])
        tmp = gpool.tile([128, KC, BS], BF16, tag=f"tmp{s}", name=f"tmp{s}")
        nc.vector.tensor_mul(tmp[:], sif[s][:, 0], tg[s][:])
        nc.vector.tensor_add(cT[s][:], cT[s][:], tmp[:])
        thc_pend[s] = so[s]

        if s == 0 and t + 3 < T:
            load_x(t + 3)

    # ---- epilogue: last tanh_c/h for stream 1, then the output head ----
    y_ps = pspool.tile([64, BL], F32, tag="pif", name="y_ps")
    for k in range(KC):   # stream 0's head: h0 is already final
        nc.tensor.matmul(y_ps[:, 0:BS], wt[:, k, :], hT[0][:, k, :],
                         start=(k == 0), stop=(k == KC - 1))
    sb = 1
    thc = gpool.tile([128, KC, BS], BF16, tag=f"th{sb}", name="thc_last")
    nc.scalar.activation(thc[:], cT[sb][:], AF.Tanh)
    nc.vector.tensor_mul(hT[sb][:], thc_pend[sb][:], thc[:])
    for k in range(KC):
        nc.tensor.matmul(y_ps[:, BS:], wt[:, k, :], hT[1][:, k, :],
                         start=(k == 0), stop=(k == KC - 1))
    y_sb = gpool.tile([64, BL], F32, tag="y")
    nc.scalar.activation(y_sb[:], y_ps[:, :], AF.Identity, bias=bb[:])
    nc.sync.dma_start(out_ext.ap(), y_sb[:])


def get_nc():
    global _NC_CACHE
    if _NC_CACHE is None:
        _NC_CACHE = build_nc()
    return _NC_CACHE


def make_in_maps(x, U_ih, V_ih, U_hh, V_hh, W_out, b_out):
    """Shard + pre-transpose the full inputs into per-core in_maps."""
    x = np.asarray(x, dtype=np.float32)
    vt = np.zeros((64, 4 * H), dtype=np.float32)
    vt[0:R, :] = np.asarray(V_ih, np.float32).T
    vt[32:32 + R, :] = np.asarray(V_hh, np.float32).T
    vt = vt.astype(ml_dtypes.bfloat16)
    shared = {
        "u_ih": np.asarray(U_ih, np.float32).astype(ml_dtypes.bfloat16),
        "u_hh": np.asarray(U_hh, np.float32).astype(ml_dtypes.bfloat16),
        "vt": vt,
        "wt": np.ascontiguousarray(np.asarray(W_out, np.float32).T).astype(
            ml_dtypes.bfloat16),
        "b_out": np.ascontiguousarray(
            np.asarray(b_out, np.float32).reshape(DO, 1)),
    }
    in_maps = []
    for c in range(N_CORES):
        xc = x[c * BL:(c + 1) * BL]              # [BL, T, D]
        xc = np.ascontiguousarray(xc.transpose(1, 2, 0)).astype(
            ml_dtypes.bfloat16)                           # [T, D, BL] bf16
        in_maps.append({"x": xc, **shared})
    return in_maps


def kernel(x, U_ih, V_ih, U_hh, V_hh, W_out, b_out):
    in_maps = make_in_maps(x, U_ih, V_ih, U_hh, V_hh, W_out, b_out)
    last_err = None
    for attempt in range(3):
        try:
            nc = get_nc()
            res = run_bass_kernel_spmd(nc, in_maps, list(range(N_CORES)))
            break
        except Exception as e:  # transient NRT device errors under axon
            last_err = e
            time.sleep(10)
    else:
        raise last_err
    # per-core out is y.T [DO, BL] -> assemble full y [B, DO]
    y = np.concatenate([np.asarray(res.results[c]["out"]).T
                        for c in range(N_CORES)], axis=0)
    return np.ascontiguousarray(y.astype(np.float32))
